# revision 1
# baseline (speedup 1.0000x reference)
"""HRR attention kernel for 8 Trainium2 NeuronCores.

Strategy (tensor-shard over heads, per sharding_hint):
  - H=16 heads, 8 cores -> 2 heads (128 embedding cols) per core.
  - Each core takes the FULL q/k/v and a 128-column slice of Wq/Wk/Wv
    (head projections are column-separable), runs the whole HRR
    bind/unbind/cosine/softmax pipeline for its 2 heads, then applies
    its 128-row slice of Wo.T to produce a partial [B,S,D] output.
  - Host sums the 8 partials and adds bo (output projection is a sum
    over head blocks, so partials add exactly).

The FFT bind/unbind stages are reformulated as matmuls so everything
lowers to dot_general on the NeuronCore tensor engine (no FFT needed):
  circconv(x, y)[j] = sum_i x[i] y[(j-i)%64]
  bind:   beta[b,h] = sum_s circconv(k_s, v_s)
        = contract(G, M)  with G = kp^T @ vp  (64x64, sum over S)
          and M[i,m,j] = 1 iff j == (i+m) % 64
  unbind: v_hat[b,s,h] = circconv(beta, qt_s) = qt_s @ C(beta)
          with C(beta)[m,j] = beta[(j-m)%64] built via the same M.
  approx_transpose: qt = qp @ P with P[i,j] = 1 iff i == (-j) % 64.
"""

import numpy as np
import jax
import jax.numpy as jnp
from functools import partial

B, S, D = 4, 2048, 1024
H, Hd = 16, 64
EPS = 1e-8
N_CORES = 8
HEADS_PER_CORE = H // N_CORES          # 2
COLS = HEADS_PER_CORE * Hd             # 128 embedding cols per core

# --- host-built constant operands (one-hot circulant tensors) ---
_i = np.arange(Hd)
# M[i,m,j] = 1 iff j == (i+m) % 64   (symmetric in i,m)
_M = np.zeros((Hd, Hd, Hd), np.float32)
_M[_i[:, None], _i[None, :], (_i[:, None] + _i[None, :]) % Hd] = 1.0
# P[i,j] = 1 iff i == (-j) % 64  (qt[j] = qp[(-j)%64])
_P = np.zeros((Hd, Hd), np.float32)
_P[(-_i) % Hd, _i] = 1.0


@partial(jax.jit, static_argnums=())
def _core_fn(q, k, v, WqTc, bqc, WkTc, bkc, WvTc, bvc, WoTc, M, P):
    """Full HRR pipeline for this core's 2 heads; returns partial [B,S,D]."""
    h = HEADS_PER_CORE
    qf = q.reshape(B * S, D)
    kf = k.reshape(B * S, D)
    vf = v.reshape(B * S, D)
    qp = (qf @ WqTc + bqc).reshape(B, S, h, Hd)
    kp = (kf @ WkTc + bkc).reshape(B, S, h, Hd)
    vp = (vf @ WvTc + bvc).reshape(B, S, h, Hd)

    # bind: G[b,h,i,m] = sum_s kp[b,s,h,i] vp[b,s,h,m]; beta = G : M
    G = jnp.einsum('bshi,bshm->bhim', kp, vp)
    beta = jnp.tensordot(G, M, axes=([2, 3], [0, 1]))          # [B,h,Hd]

    # unbind: qt = qp @ P ; Cbeta[b,h,m,j] = beta[b,h,(j-m)%64]
    qt = qp @ P
    Cbeta = jnp.einsum('bhi,imj->bhmj', beta, M)               # [B,h,Hd,Hd]
    v_hat = jnp.einsum('bshm,bhmj->bshj', qt, Cbeta)           # [B,S,h,Hd]

    # cosine similarity along Hd (torch semantics: clamp each norm at eps)
    dot = (vp * v_hat).sum(-1)
    nv = jnp.maximum(jnp.sqrt((vp * vp).sum(-1)), EPS)
    nh = jnp.maximum(jnp.sqrt((v_hat * v_hat).sum(-1)), EPS)
    a = dot / (nv * nh)                                        # [B,S,h]

    w = jax.nn.softmax(a, axis=1)                              # over S
    attn = (w[..., None] * vp).reshape(B * S, h * Hd)          # [B*S,128]
    return (attn @ WoTc).reshape(B, S, D)                      # partial


def kernel(q, k, v, Wq, bq, Wk, bk, Wv, bv, Wo, bo, **_):
    devs = jax.devices()[:N_CORES]
    q = np.ascontiguousarray(q, np.float32)
    k = np.ascontiguousarray(k, np.float32)
    v = np.ascontiguousarray(v, np.float32)

    outs = []
    for c, dev in enumerate(devs):
        sl = slice(c * COLS, (c + 1) * COLS)
        args = (
            q, k, v,
            np.ascontiguousarray(Wq[sl].T), bq[sl],
            np.ascontiguousarray(Wk[sl].T), bk[sl],
            np.ascontiguousarray(Wv[sl].T), bv[sl],
            np.ascontiguousarray(Wo[:, sl].T),
            _M, _P,
        )
        dargs = [jax.device_put(a, dev) for a in args]
        outs.append(_core_fn(*dargs))   # async dispatch; cores run in parallel

    out = np.zeros((B, S, D), np.float32)
    for o in outs:
        out += np.asarray(o)
    out += np.asarray(bo, np.float32)
    return out.astype(np.float32)



# revision 4
# speedup vs baseline: 14.4952x; 14.4952x over previous
"""HRR attention kernel for 8 Trainium2 NeuronCores (axon-tunneled).

The axon host<->device tunnel is the bottleneck (~40 MB/s H2D, ~30 MB/s
D2H, serialized across devices), so the kernel is organized to move each
byte across the tunnel exactly once, in fp16:

  - q/k/v are flattened to [B*S, D] and row-sharded over the 8 cores
    (1024 rows each). Core 2b holds batch b, s in [0,1024); core 2b+1
    holds batch b, s in [1024,2048).  H2D: 48 MB fp16 instead of the
    768 MB f32 a replicate-everything layout costs.
  - The four 1024x1024 projection weights are row-sharded (128 rows per
    core, 8 MB total fp16) and all-gathered on the device fabric.
  - The two sequence-wide reductions (bind-stage sum over S and the
    softmax over S) are cross-core psums over the core pairs that share
    a batch: [[0,1],[2,3],[4,5],[6,7]] -- a few hundred KB on fabric.
  - The output comes back as an fp16 row-sharded [B*S, D] array (16 MB).

All compute is f32 on device; fp16 is only the wire format (rel-err
budget is 2e-2, fp16 quantization contributes ~1e-3).

FFT bind/unbind are reformulated as tiny matmuls with one-hot circulant
tensors built on-device from iotas:
  circconv(x, y)[j] = sum_i x[i] y[(j-i)%64]
  bind:   beta[h,j] = sum_{i,m:(i+m)%64==j} G[h,i,m],  G = kp^T @ vp
  unbind: v_hat = qt @ C(beta), C(beta)[m,j] = beta[(j-m)%64]
  approx_transpose: qt = qp @ P, P[i,j] = 1 iff (i+j)%64 == 0.
"""

import os
import numpy as np
import jax
import jax.numpy as jnp
from jax.sharding import Mesh, NamedSharding, PartitionSpec as P
from functools import partial

try:
    from jax import shard_map
    _SM_KW = {'check_vma': False}
except ImportError:
    from jax.experimental.shard_map import shard_map
    _SM_KW = {'check_rep': False}

# persistent compile cache so a fresh process skips neuron recompiles
try:
    jax.config.update("jax_compilation_cache_dir", "/tmp/jax_comp_cache")
    jax.config.update("jax_persistent_cache_min_compile_time_secs", 1.0)
except Exception:
    pass

B, S, D = 4, 2048, 1024
H, Hd = 16, 64
EPS = 1e-8
N = 8
ROWS = B * S // N          # 1024 rows per core
PAIRS = [[0, 1], [2, 3], [4, 5], [6, 7]]  # cores sharing a batch

_mesh = None
_sh_rows = None
_sh_rep = None


def _init_mesh():
    global _mesh, _sh_rows, _sh_rep
    if _mesh is None:
        devs = jax.devices()[:N]
        _mesh = Mesh(np.array(devs), ('x',))
        _sh_rows = NamedSharding(_mesh, P('x', None))
        _sh_rep = NamedSharding(_mesh, P())
    return _mesh, _sh_rows, _sh_rep


def _core(q, k, v, Wq, bq, Wk, bk, Wv, bv, Wo, bo):
    """Per-core body. q/k/v: [ROWS, D] fp16 shard. W*: [D//N, D] fp16 shard.
    biases: [D] f32 replicated. Returns [ROWS, D] fp16 output shard."""
    # weights: all-gather shards on fabric, cast to f32
    Wq = jax.lax.all_gather(Wq, 'x', tiled=True).astype(jnp.float32)
    Wk = jax.lax.all_gather(Wk, 'x', tiled=True).astype(jnp.float32)
    Wv = jax.lax.all_gather(Wv, 'x', tiled=True).astype(jnp.float32)
    Wo = jax.lax.all_gather(Wo, 'x', tiled=True).astype(jnp.float32)

    qf = q.astype(jnp.float32)
    kf = k.astype(jnp.float32)
    vf = v.astype(jnp.float32)

    qp = (qf @ Wq.T + bq).reshape(ROWS, H, Hd)
    kp = (kf @ Wk.T + bk).reshape(ROWS, H, Hd)
    vp = (vf @ Wv.T + bv).reshape(ROWS, H, Hd)

    # one-hot circulant helpers, built on device
    i3 = jax.lax.broadcasted_iota(jnp.int32, (Hd, Hd, Hd), 0)
    m3 = jax.lax.broadcasted_iota(jnp.int32, (Hd, Hd, Hd), 1)
    j3 = jax.lax.broadcasted_iota(jnp.int32, (Hd, Hd, Hd), 2)
    M = ((i3 + m3 - j3) % Hd == 0).astype(jnp.float32)      # [Hd,Hd,Hd]
    i2 = jax.lax.broadcasted_iota(jnp.int32, (Hd, Hd), 0)
    j2 = jax.lax.broadcasted_iota(jnp.int32, (Hd, Hd), 1)
    Pm = ((i2 + j2) % Hd == 0).astype(jnp.float32)          # [Hd,Hd]

    # bind: G[h,i,m] = sum_local_s kp[s,h,i] vp[s,h,m]; psum over the pair
    G = jnp.einsum('shi,shm->him', kp, vp)
    G = jax.lax.psum(G, 'x', axis_index_groups=PAIRS)        # [H,Hd,Hd]
    beta = G.reshape(H, Hd * Hd) @ M.reshape(Hd * Hd, Hd)    # [H,Hd]

    # unbind: qt = qp @ P ; Cbeta[h,m,j] = beta[h,(j-m)%64]
    qt = jnp.einsum('shm,mj->shj', qp, Pm)
    Cbeta = (beta @ M.reshape(Hd, Hd * Hd)).reshape(H, Hd, Hd)
    v_hat = jnp.einsum('shm,hmj->shj', qt, Cbeta)            # [ROWS,H,Hd]

    # cosine similarity along Hd (clamp each norm at eps)
    dot = (vp * v_hat).sum(-1)
    nv = jnp.maximum(jnp.sqrt((vp * vp).sum(-1)), EPS)
    nh = jnp.maximum(jnp.sqrt((v_hat * v_hat).sum(-1)), EPS)
    a = dot / (nv * nh)                                      # [ROWS,H]

    # softmax over S = the two cores of this pair
    m_loc = a.max(axis=0)
    m_glob = jax.lax.pmax(m_loc, 'x', axis_index_groups=PAIRS)
    e = jnp.exp(a - m_glob)
    s_loc = e.sum(axis=0)
    s_glob = jax.lax.psum(s_loc, 'x', axis_index_groups=PAIRS)
    w = e / s_glob                                           # [ROWS,H]

    attn = (w[..., None] * vp).reshape(ROWS, D)
    out = attn @ Wo.T + bo
    return out.astype(jnp.float16)


@partial(jax.jit, static_argnums=())
def _spmd(q, k, v, Wq, bq, Wk, bk, Wv, bv, Wo, bo):
    mesh, _, _ = _init_mesh()
    f = shard_map(
        _core, mesh=mesh,
        in_specs=(P('x', None), P('x', None), P('x', None),
                  P('x', None), P(), P('x', None), P(),
                  P('x', None), P(), P('x', None), P()),
        out_specs=P('x', None),
        **_SM_KW,
    )
    return f(q, k, v, Wq, bq, Wk, bk, Wv, bv, Wo, bo)


def kernel(q, k, v, Wq, bq, Wk, bk, Wv, bv, Wo, bo, **_):
    mesh, sh_rows, sh_rep = _init_mesh()

    put = jax.device_put
    dq = put(np.asarray(q, np.float16).reshape(B * S, D), sh_rows)
    dk = put(np.asarray(k, np.float16).reshape(B * S, D), sh_rows)
    dv = put(np.asarray(v, np.float16).reshape(B * S, D), sh_rows)
    dWq = put(np.asarray(Wq, np.float16), sh_rows)
    dWk = put(np.asarray(Wk, np.float16), sh_rows)
    dWv = put(np.asarray(Wv, np.float16), sh_rows)
    dWo = put(np.asarray(Wo, np.float16), sh_rows)
    dbq = put(np.asarray(bq, np.float32), sh_rep)
    dbk = put(np.asarray(bk, np.float32), sh_rep)
    dbv = put(np.asarray(bv, np.float32), sh_rep)
    dbo = put(np.asarray(bo, np.float32), sh_rep)

    out = _spmd(dq, dk, dv, dWq, dbq, dWk, dbk, dWv, dbv, dWo, dbo)
    return np.asarray(out).astype(np.float32).reshape(B, S, D)


# revision 5
# speedup vs baseline: 25.8501x; 1.7834x over previous
"""HRR attention kernel for 8 Trainium2 NeuronCores (axon-tunneled).

The axon host<->device tunnel is the bottleneck (~40 MB/s each way,
serialized across devices, ~13 ms fixed cost per shard transfer), so the
kernel is built to minimize both wire bytes and transfer count:

  H2D: ONE uint8 payload array [8, PAY], row-sharded (one 2.2 MB
  transfer per core, 17.8 MB total) carrying:
    - q/k/v int4 (per-64-block scales), packed two nibbles per byte
    - Wq/Wk/Wv/Wo int8 (per-64-block scales), row-sharded 128 rows/core
    - all scales as uint16 fixed-point split into lo/hi uint8 planes
    - the four biases as uint16 fixed-point planes (replicated per core)
  D2H: ONE uint8 array [8192, 544]: int4 nibbles of attn @ Wo.T
  (WITHOUT bo -- the output is ~99% bo, so bo is added on host in f32
  and the quantization scale only spans the small attention part),
  plus uint16 fixed-point per-64-block scale planes.

Quantization error budget (measured against the CPU reference):
int4 qkv + int8 W -> 1.1e-3, int4 output sans bo -> 1.1e-3, device
compute ~2e-4; total ~2e-3 vs the 2e-2 gate.

Sharding: rows of the flattened [B*S=8192, D] tensors, 1024 rows/core;
core 2b holds batch b s<1024, core 2b+1 batch b s>=1024. Cross-core
reductions (bind-stage sum over S, softmax over S) are psums over core
pairs [[0,1],[2,3],[4,5],[6,7]]. Weights are all-gathered on fabric.

FFT bind/unbind are reformulated as tiny matmuls with one-hot circulant
tensors built on-device from iotas:
  circconv(x, y)[j] = sum_i x[i] y[(j-i)%64]
  bind:   beta[h,j] = sum_{i,m:(i+m)%64==j} G[h,i,m],  G = kp^T @ vp
  unbind: v_hat = qt @ C(beta), C(beta)[m,j] = beta[(j-m)%64]
  approx_transpose: qt = qp @ P, P[i,j] = 1 iff (i+j)%64 == 0.
"""

import numpy as np
import jax
import jax.numpy as jnp
from jax.sharding import Mesh, NamedSharding, PartitionSpec as P
from functools import partial

try:
    from jax import shard_map
    _SM_KW = {'check_vma': False}
except ImportError:
    from jax.experimental.shard_map import shard_map
    _SM_KW = {'check_rep': False}

try:
    jax.config.update("jax_compilation_cache_dir", "/tmp/jax_comp_cache")
    jax.config.update("jax_persistent_cache_min_compile_time_secs", 1.0)
except Exception:
    pass

B, S, D = 4, 2048, 1024
H, Hd = 16, 64
EPS = 1e-8
N = 8
ROWS = B * S // N              # 1024 rows per core
WROWS = D // N                 # 128 weight rows per core
PAIRS = [[0, 1], [2, 3], [4, 5], [6, 7]]

# fixed-point quanta for uint16-encoded scales/biases (clamped on encode)
SQ_QKV = 1e-5                  # qkv block scales ~0.38, max 0.655
SQ_W = 1e-6                    # W block scales ~0.0076, max 0.0655
SQ_B = 4e-6                    # biases ~N(0,0.02^2), max 0.131 offset-binary
SQ_OUT = 1e-6                  # output block scales << 0.0655

# per-core payload layout (offsets in bytes)
_QNIB = ROWS * (D // 2)        # 524288 per tensor
_SCL = ROWS * 32               # 32768: scale lo/hi planes [ROWS,16]+[ROWS,16]
_WNIB = WROWS * D              # 131072 per weight
_WSCL = WROWS * 32             # 4096 per weight
_BPL = 4 * 2 * D               # 8192: 4 biases, lo+hi planes
OFF_Q, OFF_K, OFF_V = 0, _QNIB, 2 * _QNIB
OFF_QS = 3 * _QNIB
OFF_KS = OFF_QS + _SCL
OFF_VS = OFF_KS + _SCL
OFF_W = OFF_VS + _SCL          # 4 weights contiguous
OFF_WS = OFF_W + 4 * _WNIB    # 4 weight-scale blocks contiguous
OFF_B = OFF_WS + 4 * _WSCL
PAY = OFF_B + _BPL             # 2220032

OUT_COLS = D // 2 + 32         # 544: nibbles + scale lo/hi planes


_mesh = None
_sh_pay = None


def _init_mesh():
    global _mesh, _sh_pay
    if _mesh is None:
        devs = jax.devices()[:N]
        _mesh = Mesh(np.array(devs), ('x',))
        _sh_pay = NamedSharding(_mesh, P('x', None))
    return _mesh, _sh_pay


# ---------------- host-side pack/unpack (jit on CPU) ----------------

_cpu = None


def _get_cpu():
    global _cpu
    if _cpu is None:
        _cpu = jax.devices('cpu')[0]
    return _cpu


def _quant_nib_host(x):
    """x [R,1024] f32 -> nibbles packed [R,512] uint8 (+8 offset, halves
    scheme: byte j = n[j] | n[512+j]<<4), scales_enc [R,32] uint8 planes."""
    xb = x.reshape(-1, H, Hd)
    am = jnp.max(jnp.abs(xb), axis=2)
    s = jnp.maximum(am / 7.0, 1e-8)                       # [R,16]
    n = jnp.clip(jnp.round(xb / s[:, :, None]), -7, 7) + 8
    n = n.reshape(-1, D).astype(jnp.uint8)
    p = n[:, :D // 2] | (n[:, D // 2:] << 4)
    senc = jnp.clip(jnp.round(s / SQ_QKV), 0, 65535).astype(jnp.uint32)
    slo = (senc & 255).astype(jnp.uint8)
    shi = (senc >> 8).astype(jnp.uint8)
    return p, jnp.concatenate([slo, shi], axis=1)


def _quant_w_host(w):
    """w [1024,1024] f32 -> int8-as-uint8 (+128) [1024,1024], scale
    planes [1024,32] uint8."""
    wb = w.reshape(-1, H, Hd)
    am = jnp.max(jnp.abs(wb), axis=2)
    s = jnp.maximum(am / 127.0, 1e-9)
    n = jnp.clip(jnp.round(wb / s[:, :, None]), -127, 127) + 128
    n = n.reshape(-1, D).astype(jnp.uint8)
    senc = jnp.clip(jnp.round(s / SQ_W), 0, 65535).astype(jnp.uint32)
    slo = (senc & 255).astype(jnp.uint8)
    shi = (senc >> 8).astype(jnp.uint8)
    return n, jnp.concatenate([slo, shi], axis=1)


@partial(jax.jit, backend='cpu')
def _pack_host(q, k, v, Wq, bq, Wk, bk, Wv, bv, Wo, bo):
    qp_, qs = _quant_nib_host(q.reshape(B * S, D))
    kp_, ks = _quant_nib_host(k.reshape(B * S, D))
    vp_, vs = _quant_nib_host(v.reshape(B * S, D))
    wn, wsc = [], []
    for w in (Wq, Wk, Wv, Wo):
        n, sc = _quant_w_host(w)
        wn.append(n)
        wsc.append(sc)
    benc = jnp.clip(jnp.round(jnp.stack([bq, bk, bv, bo]) / SQ_B) + 32768,
                    0, 65535).astype(jnp.uint32)            # [4,1024]
    blo = (benc & 255).astype(jnp.uint8)
    bhi = (benc >> 8).astype(jnp.uint8)
    bpl = jnp.concatenate([blo, bhi], axis=1)               # [4,2048]

    parts = []
    for c in range(N):
        r = slice(c * ROWS, (c + 1) * ROWS)
        wr = slice(c * WROWS, (c + 1) * WROWS)
        parts.append(jnp.concatenate([
            qp_[r].reshape(-1), kp_[r].reshape(-1), vp_[r].reshape(-1),
            qs[r].reshape(-1), ks[r].reshape(-1), vs[r].reshape(-1),
            wn[0][wr].reshape(-1), wn[1][wr].reshape(-1),
            wn[2][wr].reshape(-1), wn[3][wr].reshape(-1),
            wsc[0][wr].reshape(-1), wsc[1][wr].reshape(-1),
            wsc[2][wr].reshape(-1), wsc[3][wr].reshape(-1),
            bpl.reshape(-1),
        ]))
    return jnp.stack(parts)                                 # [8, PAY] uint8


@partial(jax.jit, backend='cpu')
def _unpack_host(pay, bo):
    """pay [B*S, 544] uint8 -> out [B,S,D] f32 (bo added here in f32)."""
    p = pay[:, :D // 2]
    nlo = (p & 15).astype(jnp.float32) - 8.0
    nhi = (p >> 4).astype(jnp.float32) - 8.0
    n = jnp.concatenate([nlo, nhi], axis=1)                 # [R,1024]
    slo = pay[:, D // 2:D // 2 + 16].astype(jnp.uint32)
    shi = pay[:, D // 2 + 16:].astype(jnp.uint32)
    s = (slo | (shi << 8)).astype(jnp.float32) * SQ_OUT     # [R,16]
    y = (n.reshape(-1, H, Hd) * s[:, :, None]).reshape(B, S, D)
    return y + bo[None, None, :].astype(jnp.float32)


# ---------------- device-side decode/compute/encode ----------------

def _dec_scales(plane, quant, rows):
    """plane [rows*32] uint8 -> scales [rows,16] f32."""
    pl = plane.reshape(rows, 32).astype(jnp.float32)
    lo, hi = pl[:, :16], pl[:, 16:]
    return (lo + hi * 256.0) * quant


def _dec_nib(pb, splane, rows):
    """packed nibbles [rows*512] uint8 + scale plane -> [rows,1024] f32."""
    p = pb.reshape(rows, D // 2).astype(jnp.float32)
    hi = jnp.floor(p * (1.0 / 16.0))
    lo = p - hi * 16.0
    n = jnp.concatenate([lo, hi], axis=1) - 8.0             # [rows,1024]
    s = _dec_scales(splane, SQ_QKV, rows)
    return (n.reshape(rows, H, Hd) * s[:, :, None]).reshape(rows, D)


def _dec_w(wb, splane):
    """weight bytes [WROWS*1024] uint8 + scales -> [WROWS,1024] f32."""
    n = wb.reshape(WROWS, D).astype(jnp.float32) - 128.0
    s = _dec_scales(splane, SQ_W, WROWS)
    return (n.reshape(WROWS, H, Hd) * s[:, :, None]).reshape(WROWS, D)


def _core(pay):
    pay = pay.reshape(PAY)

    qf = _dec_nib(pay[OFF_Q:OFF_Q + _QNIB], pay[OFF_QS:OFF_QS + _SCL], ROWS)
    kf = _dec_nib(pay[OFF_K:OFF_K + _QNIB], pay[OFF_KS:OFF_KS + _SCL], ROWS)
    vf = _dec_nib(pay[OFF_V:OFF_V + _QNIB], pay[OFF_VS:OFF_VS + _SCL], ROWS)

    Ws = []
    for t in range(4):
        w_sh = _dec_w(pay[OFF_W + t * _WNIB:OFF_W + (t + 1) * _WNIB],
                      pay[OFF_WS + t * _WSCL:OFF_WS + (t + 1) * _WSCL])
        Ws.append(jax.lax.all_gather(w_sh, 'x', tiled=True))  # [1024,1024]
    Wq, Wk, Wv, Wo = Ws

    bpl = pay[OFF_B:OFF_B + _BPL].reshape(4, 2 * D).astype(jnp.float32)
    bia = (bpl[:, :D] + bpl[:, D:] * 256.0) * SQ_B - (32768.0 * SQ_B)
    bq, bk, bv, _bo = bia[0], bia[1], bia[2], bia[3]        # bo added on host

    qp = (qf @ Wq.T + bq).reshape(ROWS, H, Hd)
    kp = (kf @ Wk.T + bk).reshape(ROWS, H, Hd)
    vp = (vf @ Wv.T + bv).reshape(ROWS, H, Hd)

    # one-hot circulant helpers, built on device
    i3 = jax.lax.broadcasted_iota(jnp.int32, (Hd, Hd, Hd), 0)
    m3 = jax.lax.broadcasted_iota(jnp.int32, (Hd, Hd, Hd), 1)
    j3 = jax.lax.broadcasted_iota(jnp.int32, (Hd, Hd, Hd), 2)
    M = ((i3 + m3 - j3) % Hd == 0).astype(jnp.float32)
    i2 = jax.lax.broadcasted_iota(jnp.int32, (Hd, Hd), 0)
    j2 = jax.lax.broadcasted_iota(jnp.int32, (Hd, Hd), 1)
    Pm = ((i2 + j2) % Hd == 0).astype(jnp.float32)

    # bind: G[h,i,m] = sum_local_s kp[s,h,i] vp[s,h,m]; psum over the pair
    G = jnp.einsum('shi,shm->him', kp, vp)
    G = jax.lax.psum(G, 'x', axis_index_groups=PAIRS)
    beta = G.reshape(H, Hd * Hd) @ M.reshape(Hd * Hd, Hd)    # [H,Hd]

    # unbind: qt = qp @ P ; Cbeta[h,m,j] = beta[h,(j-m)%64]
    qt = jnp.einsum('shm,mj->shj', qp, Pm)
    Cbeta = (beta @ M.reshape(Hd, Hd * Hd)).reshape(H, Hd, Hd)
    v_hat = jnp.einsum('shm,hmj->shj', qt, Cbeta)            # [ROWS,H,Hd]

    # cosine similarity along Hd (clamp each norm at eps)
    dot = (vp * v_hat).sum(-1)
    nv = jnp.maximum(jnp.sqrt((vp * vp).sum(-1)), EPS)
    nh = jnp.maximum(jnp.sqrt((v_hat * v_hat).sum(-1)), EPS)
    a = dot / (nv * nh)                                      # [ROWS,H]

    # softmax over S = the two cores of this pair
    m_loc = a.max(axis=0)
    m_glob = jax.lax.pmax(m_loc, 'x', axis_index_groups=PAIRS)
    e = jnp.exp(a - m_glob)
    s_loc = e.sum(axis=0)
    s_glob = jax.lax.psum(s_loc, 'x', axis_index_groups=PAIRS)
    w = e / s_glob                                           # [ROWS,H]

    attn = (w[..., None] * vp).reshape(ROWS, D)
    y = attn @ Wo.T                                          # NO bo here

    # int4 encode with per-64-block scales, uint16 fixed-point planes
    yb = y.reshape(ROWS, H, Hd)
    am = jnp.max(jnp.abs(yb), axis=2)
    s = jnp.clip(am / 7.0, SQ_OUT, 65535.0 * SQ_OUT)         # [ROWS,16]
    n = jnp.clip(jnp.round(yb / s[:, :, None]), -7.0, 7.0) + 8.0
    n = n.reshape(ROWS, D)
    pnib = (n[:, :D // 2] + 16.0 * n[:, D // 2:]).astype(jnp.uint8)
    senc = jnp.round(s * (1.0 / SQ_OUT))
    shi = jnp.floor(senc * (1.0 / 256.0))
    slo = senc - shi * 256.0
    return jnp.concatenate([pnib, slo.astype(jnp.uint8),
                            shi.astype(jnp.uint8)], axis=1)  # [ROWS,544]


@jax.jit
def _spmd(pay):
    mesh, _ = _init_mesh()
    f = shard_map(_core, mesh=mesh, in_specs=(P('x', None),),
                  out_specs=P('x', None), **_SM_KW)
    return f(pay)


def kernel(q, k, v, Wq, bq, Wk, bk, Wv, bv, Wo, bo, **_):
    mesh, sh_pay = _init_mesh()
    cpu = _get_cpu()

    host_args = [jax.device_put(np.asarray(a, np.float32), cpu)
                 for a in (q, k, v, Wq, bq, Wk, bk, Wv, bv, Wo, bo)]
    pay = np.asarray(_pack_host(*host_args))                 # [8,PAY] uint8

    dpay = jax.device_put(pay, sh_pay)
    out_pay = _spmd(dpay)
    out_np = np.asarray(out_pay)                             # [8192,544]

    out = _unpack_host(jax.device_put(out_np, cpu),
                       jax.device_put(np.asarray(bo, np.float32), cpu))
    return np.asarray(out)
